# revision 1
# baseline (speedup 1.0000x reference)
"""Trainium2 Bass kernel for nn_ContrastiveLoss (segment_reduce).

Strategy (8 NeuronCores, SPMD):
  Phase 1: shard (batch r in 0..3) x (pixel-half). Each core computes the raw
    masked segment sums S_raw[q, ch] = sum_p combT[p, q] * feat[r, ch, p] for
    its 50 objects (rows i = q*4+r of the reference's N=200) over its pixel
    range, for both features_q and features_k, via PE matmuls contracting over
    pixels (fp32r). Features are transposed on-chip 128x128 via PE transpose.
  Gather: host concatenates per-core partial outputs (pure data movement).
  Phase 2: single core sums the two pixel-half partials, normalizes rows
    (the reference's /cnt cancels inside l2norm and pad), forms the 200x200
    logit matrix, and reduces to the contrastive loss scalar.
"""

import numpy as np
from contextlib import ExitStack

import concourse.bass as bass
import concourse.tile as tile
from concourse import bacc, mybir
from concourse.bass_utils import run_bass_kernel_spmd

# Problem constants (hardcoded per task spec)
B, M, C, H, W = 4, 50, 256, 100, 352
HW = H * W                  # 35200
N = B * M                   # 200
TAU = 0.07

P = 128                     # partitions / pixel tile
Q = M                       # 50 objects per batch
T = 138                     # pixel tiles per core (padded: 275 total = 138+137)
PX = T * P                  # 17664
CT = 23                     # pixel tiles per DMA chunk
NCHUNK = T // CT            # 6
F32R = mybir.dt.float32r
F32 = mybir.dt.float32
FP8 = mybir.dt.float8e4
NP_FP8 = mybir.dt.np(FP8)


# Force exp/ln to resolve to the combined "natural_log_exp_and_others" table
# set (index 6) instead of alternating single-function sets: empty the earlier
# sets we never want so first-match lands on sqrt_and_others (3) for
# sqrt/copy and natural_log_exp_and_others (6) for exp+ln. Indices are
# preserved so act_func_set_id stays aligned with act_info.json.
import concourse.bacc as _bacc_mod
import concourse.hw_specs as _hw_specs
_orig_get_tables = _hw_specs.get_activation_tables

def _patched_get_tables(module_arch):
    tables = dict(_orig_get_tables(module_arch))
    for i, k in enumerate(tables):
        if i in (0, 1, 2, 4, 5):
            tables[k] = set()
    return tables

_bacc_mod.get_activation_tables = _patched_get_tables

_cache = {}



def _build_phase1():
    nc = bacc.Bacc(None, target_bir_lowering=False, debug=False)
    with tile.TileContext(nc) as tc, ExitStack() as ctx:
        dram = ctx.enter_context(tc.tile_pool(name="dram", bufs=1, space="DRAM"))
        fq = dram.tile([C, PX], F32R, kind="ExternalInput", name="fq", uniquify=False)
        fk = dram.tile([C, PX], F32R, kind="ExternalInput", name="fk", uniquify=False)
        mat = dram.tile([P, T, Q], FP8, kind="ExternalInput", name="mat", uniquify=False)
        mbt = dram.tile([P, T, Q], FP8, kind="ExternalInput", name="mbt", uniquify=False)
        outq = dram.tile([Q, C], F32, kind="ExternalOutput", name="outq", uniquify=False)
        outk = dram.tile([Q, C], F32, kind="ExternalOutput", name="outk", uniquify=False)

        consts = ctx.enter_context(tc.tile_pool(name="consts", bufs=1))
        ident = consts.tile([P, P], F32)
        nc.gpsimd.memset(ident[:], 0.0)
        nc.gpsimd.affine_select(
            out=ident.bitcast(F32R), in_=ident.bitcast(F32R),
            compare_op=mybir.AluOpType.not_equal, fill=1.0, base=0,
            pattern=[[-1, P]], channel_multiplier=1)

        mask_pool = ctx.enter_context(tc.tile_pool(name="mask", bufs=1))
        CHUNKS = [6, 12, 16, 16, 16, 16, 16, 16, 16, 4, 4]
        assert sum(CHUNKS) == T
        C0 = CHUNKS[0]
        mat_sb0 = mask_pool.tile([P, C0, Q], FP8, name="mat_sb0")
        mbt_sb0 = mask_pool.tile([P, C0, Q], FP8, name="mbt_sb0")
        mat_sb = mask_pool.tile([P, T - C0, Q], FP8, name="mat_sb")
        mbt_sb = mask_pool.tile([P, T - C0, Q], FP8, name="mbt_sb")
        # chunk-0 masks land first (tiny), before any feature data
        nc.sync.dma_start(out=mat_sb0, in_=mat[:, 0:C0, :])
        nc.sync.dma_start(out=mbt_sb0, in_=mbt[:, 0:C0, :])

        psum_acc = ctx.enter_context(tc.tile_pool(name="psum_acc", bufs=1, space="PSUM"))
        ps = {"q": psum_acc.tile([Q, C], F32, name="ps_q"),
              "k": psum_acc.tile([Q, C], F32, name="ps_k")}

        fpools = {}
        for f in "qk":
            for cb in range(2):
                fpools[(f, cb)] = ctx.enter_context(
                    tc.tile_pool(name=f"f{f}{cb}", bufs=3))
        comb_pool = ctx.enter_context(tc.tile_pool(name="comb", bufs=4))
        featT_pool = ctx.enter_context(tc.tile_pool(name="featT", bufs=14))
        psum_t = ctx.enter_context(tc.tile_pool(name="psum_t", bufs=6, space="PSUM"))

        drams = {"q": fq, "k": fk}
        t0 = 0
        for chi, CTc in enumerate(CHUNKS):
            chunk = {}
            for f in "qk":
                for cb in range(2):
                    tl = fpools[(f, cb)].tile([P, CTc * P], F32R, name=f"f{f}{cb}t")
                    nc.sync.dma_start(
                        out=tl, in_=drams[f][cb * P:(cb + 1) * P, t0 * P:(t0 + CTc) * P])
                    chunk[(f, cb)] = tl
            if chi == 0:
                # remaining masks stream in behind the first feature chunk
                nc.sync.dma_start(out=mat_sb, in_=mat[:, C0:, :])
                nc.sync.dma_start(out=mbt_sb, in_=mbt[:, C0:, :])
            comb = comb_pool.tile([P, CTc, Q], F32R, name="comb")
            if chi == 0:
                nc.vector.tensor_mul(comb, mat_sb0, mbt_sb0)
            else:
                nc.vector.tensor_mul(comb, mat_sb[:, t0 - C0:t0 - C0 + CTc, :],
                                     mbt_sb[:, t0 - C0:t0 - C0 + CTc, :])
            for tt in range(CTc):
                t = t0 + tt
                for fi, f in enumerate("qk"):
                    ftT = featT_pool.tile([P, C], F32R, name="ftT")
                    pt = psum_t.tile([P, C], F32, name="pt")
                    for cb in range(2):
                        nc.tensor.transpose(
                            pt[:, cb * P:(cb + 1) * P].bitcast(F32R),
                            chunk[(f, cb)][:, tt * P:(tt + 1) * P],
                            ident.bitcast(F32R))
                    nc.vector.tensor_copy(ftT[:, :P], pt[:, :P].bitcast(F32R))
                    nc.scalar.copy(ftT[:, P:], pt[:, P:].bitcast(F32R))
                    nc.tensor.matmul(ps[f], comb[:, tt, :], ftT,
                                     start=(t == 0), stop=(t == T - 1))
            t0 += CTc

        out_pool = ctx.enter_context(tc.tile_pool(name="outp", bufs=1))
        for f, od in (("q", outq), ("k", outk)):
            o = out_pool.tile([Q, C], F32, name=f"o{f}")
            nc.vector.tensor_copy(o, ps[f])
            nc.sync.dma_start(out=od[:], in_=o)
    nc.compile()
    return nc


def _build_phase2():
    nc = bacc.Bacc(None, target_bir_lowering=False, debug=False)
    with tile.TileContext(nc) as tc, ExitStack() as ctx:
        dram = ctx.enter_context(tc.tile_pool(name="dram", bufs=1, space="DRAM"))
        pq = dram.tile([8, Q, C], F32, kind="ExternalInput", name="pq", uniquify=False)
        pk = dram.tile([8, Q, C], F32, kind="ExternalInput", name="pk", uniquify=False)
        out = dram.tile([1, 1], F32, kind="ExternalOutput", name="loss", uniquify=False)

        sb = ctx.enter_context(tc.tile_pool(name="sb", bufs=1))
        psum = ctx.enter_context(tc.tile_pool(name="psum", bufs=3, space="PSUM"))
        psum_nd = ctx.enter_context(tc.tile_pool(name="psum_nd", bufs=1, space="PSUM"))

        ident = sb.tile([P, P], F32)
        nc.gpsimd.memset(ident[:], 0.0)
        nc.gpsimd.affine_select(
            out=ident[:], in_=ident[:],
            compare_op=mybir.AluOpType.not_equal, fill=1.0, base=0,
            pattern=[[-1, P]], channel_multiplier=1)
        ones = sb.tile([P, P], F32)
        nc.gpsimd.memset(ones[:], 1.0)

        # Prefetch the sqrt table set during the input DMA (no data deps)
        warm = sb.tile([1, 1], F32)
        nc.scalar.sqrt(warm, ones[0:1, 0:1])

        # Load partials per (feature, batch r): (50-part, 2 halves, ch)
        raw = {}
        for nm, dt_ in (("q", pq), ("k", pk)):
            rt = sb.tile([Q, 8, C], F32, name=f"raw{nm}")
            for r in range(4):
                nc.sync.dma_start(out=rt[:, 2 * r:2 * r + 2, :],
                                  in_=dt_[2 * r:2 * r + 2].rearrange("e q c -> q e c"))
            raw[nm] = rt

        # Transpose-and-sum the two pixel-half partials directly in PSUM:
        # ST[nm][cb]: (128ch, 200) with column order i' = r*50+q
        ST = {}
        ncopy = 0
        for nm in "qk":
            for cb in range(2):
                stt = sb.tile([P, N], F32, name=f"ST{nm}{cb}")
                for r in range(4):
                    ptt = psum.tile([P, Q], F32, name="ptt", tag="ps")
                    for hf in range(2):
                        nc.tensor.matmul(
                            ptt, raw[nm][:, 2 * r + hf, cb * P:(cb + 1) * P],
                            ident[0:Q, 0:Q], is_transpose=True,
                            start=(hf == 0), stop=(hf == 1))
                    if ncopy % 2 == 0:
                        nc.vector.tensor_copy(stt[:, r * Q:(r + 1) * Q], ptt)
                    else:
                        nc.scalar.copy(stt[:, r * Q:(r + 1) * Q], ptt)
                    ncopy += 1
                ST[(nm, cb)] = stt

        # Row norms -> inv_k (scaled by 1/TAU), inv_q as (1, 200) rows
        inv = {}
        for nm in "qk":
            ps_n = psum.tile([1, N], F32, name="ps_n", tag="ps")
            for cb in range(2):
                sq_ = sb.tile([P, N], F32, name="sq_")
                nc.vector.tensor_mul(sq_, ST[(nm, cb)], ST[(nm, cb)])
                nc.tensor.matmul(ps_n, ones[:, 0:1], sq_,
                                 start=(cb == 0), stop=(cb == 1))
            nrm = sb.tile([1, N], F32, name=f"nrm{nm}")
            nc.scalar.sqrt(nrm, ps_n)
            nc.vector.tensor_scalar_max(nrm, nrm, 1e-12)
            iv = sb.tile([1, N], F32, name=f"inv{nm}")
            nc.vector.reciprocal(iv, nrm)
            inv[nm] = iv
        invk_tau = sb.tile([1, N], F32)
        nc.vector.tensor_scalar_mul(invk_tau, inv["k"], 1.0 / TAU)
        warm2 = sb.tile([1, 1], F32)
        nc.scalar.activation(warm2, inv["k"][:, 0:1],
                             mybir.ActivationFunctionType.Exp)

        # Broadcast col scales: Bb (128, 200) = ones_col @ inv_q
        ps_b = psum.tile([P, N], F32, name="ps_b", tag="ps")
        nc.tensor.matmul(ps_b, ones[0:1, :], inv["q"], start=True, stop=True)
        Bb = sb.tile([P, N], F32)
        nc.vector.tensor_copy(Bb, ps_b)

        # Diag row: d0[j] = sum_ch SkT[ch,j]*SqT[ch,j]; then scale
        ps_d = psum.tile([1, N], F32, name="ps_d", tag="ps")
        for cb in range(2):
            dk = sb.tile([P, N], F32, name="dk")
            nc.vector.tensor_mul(dk, ST[("k", cb)], ST[("q", cb)])
            nc.tensor.matmul(ps_d, ones[:, 0:1], dk, start=(cb == 0), stop=(cb == 1))
        drow = sb.tile([1, N], F32)
        nc.vector.tensor_mul(drow, ps_d, invk_tau)
        nc.vector.tensor_mul(drow, drow, inv["q"])

        # pad row: SkT[0, :] != 0
        padrow = sb.tile([1, N], F32)
        nc.vector.tensor_scalar(padrow, ST[("k", 0)][0:1, :], 0.0, None,
                                op0=mybir.AluOpType.not_equal)

        # Per row-block m: logits, lse, ce, masked sums
        nd_ps = psum_nd.tile([1, 2], F32, name="nd_ps")
        blocks = [(0, P), (P, N - P)]  # (start, rows)
        for mi, (i0, rows) in enumerate(blocks):
            ps_L = psum.tile([P, N], F32, name="ps_L", tag="ps")
            for cb in range(2):
                nc.tensor.matmul(ps_L[:rows, :], ST[("k", cb)][:, i0:i0 + rows],
                                 ST[("q", cb)], start=(cb == 0), stop=(cb == 1))
            # per-row scale a_i = invk_tau[i] as column
            acol_ps = psum.tile([P, 1], F32, name="acol_ps", tag="ps")
            nc.tensor.transpose(acol_ps[:rows, :], invk_tau[:, i0:i0 + rows], ident[0:1, 0:1])
            acol = sb.tile([P, 1], F32, name="acol")
            nc.vector.tensor_copy(acol[:rows], acol_ps[:rows])
            # logits = (raw * a_i) * b_j  in one fused DVE op
            lg = sb.tile([P, N], F32, name="lg")
            nc.vector.scalar_tensor_tensor(lg[:rows], ps_L[:rows, :], acol[:rows],
                                           Bb[:rows], op0=mybir.AluOpType.mult,
                                           op1=mybir.AluOpType.mult)
            # lse without max subtraction (|logits| <= ~14.3 is exp-safe)
            es = sb.tile([P, N], F32, name="es")
            ssum = sb.tile([P, 1], F32, name="ssum")
            nc.scalar.activation(es[:rows], lg[:rows],
                                 mybir.ActivationFunctionType.Exp,
                                 accum_out=ssum[:rows])
            lse = sb.tile([P, 1], F32, name="lse")
            nc.scalar.activation(lse[:rows], ssum[:rows],
                                 mybir.ActivationFunctionType.Ln)

            # diag + pad as columns (two K=1 transposes)
            d_ps = psum.tile([P, 1], F32, name="d_ps", tag="ps")
            nc.tensor.transpose(d_ps[:rows, :], drow[:, i0:i0 + rows], ident[0:1, 0:1])
            p_ps = psum.tile([P, 1], F32, name="p_ps", tag="ps")
            nc.tensor.transpose(p_ps[:rows, :], padrow[:, i0:i0 + rows], ident[0:1, 0:1])
            dcol = sb.tile([P, 1], F32, name="dcol")
            nc.vector.tensor_copy(dcol[:rows], d_ps[:rows])
            pcol = sb.tile([P, 1], F32, name="pcol")
            nc.vector.tensor_copy(pcol[:rows], p_ps[:rows])

            ce = sb.tile([P, 2], F32, name="ce")
            # ce[:,0] = (lse - d) * pad ; ce[:,1] = pad
            nc.vector.scalar_tensor_tensor(ce[:rows, 0:1], lse[:rows], dcol[:rows],
                                           pcol[:rows], op0=mybir.AluOpType.subtract,
                                           op1=mybir.AluOpType.mult)
            nc.vector.tensor_copy(ce[:rows, 1:2], pcol[:rows])
            nc.tensor.matmul(nd_ps, ones[:rows, 0:1], ce[:rows],
                             start=(mi == 0), stop=(mi == 1))

        den = sb.tile([1, 1], F32)
        nc.vector.tensor_scalar_max(den, nd_ps[:, 1:2], 1.0)
        rden = sb.tile([1, 1], F32)
        nc.vector.reciprocal(rden, den)
        res = sb.tile([1, 1], F32)
        nc.vector.tensor_mul(res, nd_ps[:, 0:1], rden)
        nc.sync.dma_start(out=out[:], in_=res)
    nc.compile()
    return nc


def _host_prep(features_q, features_k, pos_region_ranges):
    """Shard inputs (pure slicing / layout permutation / dtype packing)."""
    fq = np.ascontiguousarray(np.asarray(features_q, dtype=np.float32)).reshape(B, C, HW)
    fk = np.ascontiguousarray(np.asarray(features_k, dtype=np.float32)).reshape(B, C, HW)
    mask = np.asarray(pos_region_ranges).astype(bool).reshape(B, M, HW)
    mask_flat = mask.reshape(N, HW)

    in_maps = []
    for core in range(8):
        r, half = core // 2, core % 2
        lo = half * PX
        hi = min(lo + PX, HW)
        n = hi - lo

        def shard_feat(f):
            out = np.zeros((C, PX), np.float32)
            out[:, :n] = f[r, :, lo:hi]
            return out

        def shard_mask(rows):  # rows: (50, HW) bool
            t = np.zeros((Q, PX), NP_FP8)
            t[:, :n] = rows[:, lo:hi].astype(NP_FP8)
            # (50, T*128) -> (50, T, 128) -> (128, T, 50)
            return np.ascontiguousarray(t.reshape(Q, T, P).transpose(2, 1, 0))

        in_maps.append({
            "fq": shard_feat(fq),
            "fk": shard_feat(fk),
            "mat": shard_mask(mask_flat[r::4]),      # mA rows i = q*4+r
            "mbt": shard_mask(mask[r]),              # mB rows = mask[r, q]
        })
    return in_maps


def kernel(features_q, features_k, pos_region_ranges):
    if "p1" not in _cache:
        _cache["p1"] = _build_phase1()
        _cache["p2"] = _build_phase2()
    nc1, nc2 = _cache["p1"], _cache["p2"]

    in_maps = _host_prep(features_q, features_k, pos_region_ranges)
    r1 = run_bass_kernel_spmd(nc1, in_maps, core_ids=list(range(8)))

    pq = np.stack([r1.results[i]["outq"] for i in range(8)])  # (8, 50, 256)
    pk = np.stack([r1.results[i]["outk"] for i in range(8)])
    r2 = run_bass_kernel_spmd(nc2, [{"pq": pq, "pk": pk}], core_ids=[0])
    loss = r2.results[0]["loss"][0, 0]
    return np.float32(loss)

